# revision 5
# baseline (speedup 1.0000x reference)
"""Gaussian-splat attention (MinimalGSA) on 8 Trainium2 cores, head-parallel.

Self-contained: builds a Bass/Tile kernel per head (one head per NeuronCore),
runs SPMD via PJRT/axon, assembles full outputs on host.

Per-core (head h) device pipeline, all matmuls in float32r (TF32):
  A: qkvT[f,t] = Wqkv_h^T @ x^T            (x^T streamed, t-quartered psum)
  B: v -> natural layout via PE transpose; t_sq via DVE square + ones-matmul;
     qaT/kwT[s,t] = Exp(psum(2*inv2s2*dots - inv2s2*t_sq) + bias[s])
     with bias folding c_sq, log_amplitudes; cdist clamp dropped (d2>=0 up to
     ~1e-7 rounding, harmless under exp).
  C: per (b, i-chunk of 512):
     nat:  scores[i,j] (K=32) -> Exp(scale=1/temp, accum_out=denom)
           -> att = un * recip -> DMA
     T:    scoresT[j,i] -> Exp -> attT ; AV: outT[d,i] += v_nat^T @ attT
     proj: partial[i,:] = (outT[:,i]^T @ Wout_h) * recip[i] -> DMA
Host: attention[...,h] = att_h ; final = sum_h partial_h + bout.
"""
import contextlib

import numpy as np

import concourse.bacc as bacc
import concourse.mybir as mybir
import concourse.tile as tile
from concourse.bass import ts as _ts

F32 = mybir.dt.float32
F32R = mybir.dt.float32r
AF = mybir.ActivationFunctionType

B, T, D, H, S = 2, 2048, 1024, 8, 32
Dh = D // H
BT = B * T
EPS = 1e-6


def build_kernel(repeat=1):
    nc = bacc.Bacc("TRN2", target_bir_lowering=False, debug=False)

    xT = nc.dram_tensor("xT", [D, BT], F32R, kind="ExternalInput").ap()
    wqkv = nc.dram_tensor("wqkv", [D, 3 * Dh], F32R, kind="ExternalInput").ap()
    bqkv = nc.dram_tensor("bqkv", [Dh, 3], F32, kind="ExternalInput").ap()
    cqs = nc.dram_tensor("cqs", [Dh, S], F32R, kind="ExternalInput").ap()
    neginv = nc.dram_tensor("neginv", [1, S], F32R, kind="ExternalInput").ap()
    biasq = nc.dram_tensor("biasq", [S, 1], F32, kind="ExternalInput").ap()
    biask = nc.dram_tensor("biask", [S, 1], F32, kind="ExternalInput").ap()
    invt = nc.dram_tensor("invt", [128, 1], F32, kind="ExternalInput").ap()
    ones_col = nc.dram_tensor("ones_col", [Dh, 1], F32R, kind="ExternalInput").ap()
    ident = nc.dram_tensor("ident", [128, 128], F32R, kind="ExternalInput").ap()
    wout = nc.dram_tensor("wout", [Dh, D], F32R, kind="ExternalInput").ap()

    att_out = nc.dram_tensor("att", [B, T, T], F32, kind="ExternalOutput").ap()
    partial = nc.dram_tensor("partial", [BT, D], F32, kind="ExternalOutput").ap()

    with tile.TileContext(nc) as tc, contextlib.ExitStack() as big:
        const = big.enter_context(tc.tile_pool(name="const", bufs=1))
        w_sb = const.tile([128, D // 128, 3 * Dh], F32R)
        bqkv_sb = const.tile([Dh, 3], F32)
        cqs_sb = const.tile([Dh, S], F32R)
        neginv_sb = const.tile([1, S], F32R)
        biasq_sb = const.tile([S, 1], F32)
        biask_sb = const.tile([S, 1], F32)
        invt_sb = const.tile([128, 1], F32)
        ones_sb = const.tile([Dh, 1], F32R)
        ident_sb = const.tile([128, 128], F32R)
        wout_sb = const.tile([Dh, D], F32R)
        for kt in range(D // 128):
            nc.sync.dma_start(w_sb[:, kt, :], wqkv[_ts(kt, 128), :])
        nc.sync.dma_start(bqkv_sb[:], bqkv[:])
        nc.sync.dma_start(cqs_sb[:], cqs[:])
        nc.sync.dma_start(neginv_sb[:], neginv[:])
        nc.sync.dma_start(biasq_sb[:], biasq[:])
        nc.sync.dma_start(biask_sb[:], biask[:])
        nc.sync.dma_start(invt_sb[:], invt[:])
        nc.sync.dma_start(ones_sb[:], ones_col[:])
        nc.sync.dma_start(ident_sb[:], ident[:])
        nc.sync.dma_start(wout_sb[:], wout[:])

        for _rep in range(repeat):
            with contextlib.ExitStack() as rep_stack:
                keep = rep_stack.enter_context(
                    tc.tile_pool(name="keep", bufs=1))
                v_nat = keep.tile([128, BT], F32R)   # [t-part, (b,jt) d-blocks]
                qaT = keep.tile([S, BT], F32R)
                kwT = keep.tile([S, BT], F32R)
                recip_all = keep.tile([128, 32], F32)
                outT = keep.tile([Dh, BT], F32R)

                with contextlib.ExitStack() as pab:
                    qkvp = pab.enter_context(
                        tc.tile_pool(name="qkvT", bufs=1))
                    qT = qkvp.tile([Dh, BT], F32R, tag="qT")
                    kT = qkvp.tile([Dh, BT], F32R, tag="kT")
                    vT = qkvp.tile([Dh, BT], F32R, tag="vT")
                    dst = {0: qT, 1: kT, 2: vT}

                    # ---------------- Phase A: qkvT ----------------
                    with contextlib.ExitStack() as pa:
                        xpool = pa.enter_context(
                            tc.tile_pool(name="xT", bufs=3))
                        ps_a = pa.enter_context(
                            tc.tile_pool(name="ps_a", bufs=1, space="PSUM"))
                        for tq in range(4):
                            pss = {}
                            for f in range(3):
                                for c in range(2):
                                    pss[f, c] = ps_a.tile(
                                        [128, 512], F32, tag=f"psa{f}{c}",
                                        name=f"psa{f}{c}")
                            for kt in range(8):
                                xt = xpool.tile([128, 1024], F32R, tag="xt")
                                nc.sync.dma_start(
                                    xt[:], xT[_ts(kt, 128), _ts(tq, 1024)])
                                for f in range(3):
                                    for c in range(2):
                                        nc.tensor.matmul(
                                            pss[f, c][:],
                                            w_sb[:, kt, _ts(f, 128)],
                                            xt[:, _ts(c, 512)],
                                            start=(kt == 0), stop=(kt == 7))
                            for f in range(3):
                                for c in range(2):
                                    off = tq * 1024 + c * 512
                                    nc.vector.tensor_scalar_add(
                                        dst[f][:, off:off + 512],
                                        pss[f, c][:], bqkv_sb[:, f:f + 1])

                    # ------- Phase B: v_nat, t_sq, splat weights -------
                    with contextlib.ExitStack() as pb:
                        ps_t = pb.enter_context(
                            tc.tile_pool(name="ps_t", bufs=2, space="PSUM"))
                        sqp = pb.enter_context(
                            tc.tile_pool(name="sq", bufs=2))
                        tsqp = pb.enter_context(
                            tc.tile_pool(name="tsq", bufs=1))
                        for g in range(8):
                            pst = ps_t.tile([128, 512], F32R, tag="pst")
                            for j in range(4):
                                blk = g * 4 + j
                                nc.tensor.transpose(
                                    pst[:, _ts(j, 128)],
                                    vT[:, _ts(blk, 128)], ident_sb[:])
                            nc.vector.tensor_copy(v_nat[:, _ts(g, 512)], pst[:])
                        tsq_q = tsqp.tile([1, BT], F32R, tag="tsq_q")
                        tsq_k = tsqp.tile([1, BT], F32R, tag="tsq_k")
                        tsqs = {0: tsq_q, 1: tsq_k}
                        for row, src in ((0, qT), (1, kT)):
                            for ch in range(8):
                                sq = sqp.tile([Dh, 512], F32R, tag="sqc")
                                nc.vector.tensor_mul(
                                    sq[:], src[:, _ts(ch, 512)],
                                    src[:, _ts(ch, 512)])
                                ps1 = ps_t.tile([1, 512], F32, tag="ps1")
                                nc.tensor.matmul(
                                    ps1[:], ones_sb[:], sq[:],
                                    start=True, stop=True)
                                nc.vector.tensor_copy(
                                    tsqs[row][:, _ts(ch, 512)], ps1[:])
                        for row, src, tgt, bias in (
                                (0, qT, qaT, biasq_sb),
                                (1, kT, kwT, biask_sb)):
                            for ch in range(8):
                                ps32 = ps_t.tile([S, 512], F32, tag="ps32")
                                nc.tensor.matmul(
                                    ps32[:], cqs_sb[:], src[:, _ts(ch, 512)],
                                    start=True, stop=False)
                                nc.tensor.matmul(
                                    ps32[:], neginv_sb[:],
                                    tsqs[row][:, _ts(ch, 512)],
                                    start=False, stop=True)
                                nc.scalar.activation(
                                    tgt[:, _ts(ch, 512)], ps32[:], AF.Exp,
                                    bias=bias[:], scale=1.0)

                # ---------------- Phase C: attention ----------------
                with contextlib.ExitStack() as pc:
                    ps_nat = pc.enter_context(
                        tc.tile_pool(name="ps_nat", bufs=2, space="PSUM"))
                    ps_T = pc.enter_context(
                        tc.tile_pool(name="ps_T", bufs=2, space="PSUM"))
                    ps_av = pc.enter_context(
                        tc.tile_pool(name="ps_av", bufs=1, space="PSUM"))
                    ps_pr = pc.enter_context(
                        tc.tile_pool(name="ps_pr", bufs=1, space="PSUM"))
                    unp = pc.enter_context(tc.tile_pool(name="att_un", bufs=3))
                    stp = pc.enter_context(tc.tile_pool(name="att_st", bufs=3))
                    tTp = pc.enter_context(tc.tile_pool(name="attT", bufs=20))
                    denp = pc.enter_context(tc.tile_pool(name="den", bufs=8))
                    finp = pc.enter_context(tc.tile_pool(name="fin", bufs=3))

                    for b in range(B):
                        for ic in range(4):
                            for it in range(4):
                                i_tile = ic * 4 + it
                                col = b * 16 + i_tile
                                i0 = b * T + i_tile * 128
                                un = unp.tile([128, T], F32R, tag="un")
                                den = denp.tile([128, 2], F32, tag="den")
                                for jh in range(2):
                                    ps = ps_nat.tile([128, 1024], F32,
                                                     tag="psn")
                                    for jc in range(2):
                                        j0 = b * T + jh * 1024 + jc * 512
                                        nc.tensor.matmul(
                                            ps[:, _ts(jc, 512)],
                                            qaT[:, i0:i0 + 128],
                                            kwT[:, j0:j0 + 512],
                                            start=True, stop=True)
                                    nc.scalar.activation(
                                        un[:, _ts(jh, 1024)], ps[:], AF.Exp,
                                        bias=0.0, scale=invt_sb[:],
                                        accum_out=den[:, jh:jh + 1])
                                dsum = denp.tile([128, 1], F32, tag="dsum")
                                nc.vector.tensor_reduce(
                                    dsum[:], den[:],
                                    axis=mybir.AxisListType.X,
                                    op=mybir.AluOpType.add)
                                nc.vector.reciprocal(
                                    recip_all[:, col:col + 1], dsum[:])
                                st = stp.tile([128, T], F32, tag="st")
                                nc.vector.tensor_scalar_mul(
                                    st[:], un[:], recip_all[:, col:col + 1])
                                nc.sync.dma_start(
                                    att_out[b, _ts(i_tile, 128), :], st[:])
                            # transposed side + AV
                            psav = ps_av.tile([Dh, 512], F32, tag="psav")
                            for jt in range(16):
                                psT = ps_T.tile([128, 512], F32, tag="psT")
                                nc.tensor.matmul(
                                    psT[:],
                                    kwT[:, b * T + jt * 128:
                                        b * T + jt * 128 + 128],
                                    qaT[:, b * T + ic * 512:
                                        b * T + ic * 512 + 512],
                                    start=True, stop=True)
                                aT = tTp.tile([128, 512], F32R, tag="aT")
                                nc.scalar.activation(
                                    aT[:], psT[:], AF.Exp,
                                    bias=0.0, scale=invt_sb[:])
                                nc.tensor.matmul(
                                    psav[:],
                                    v_nat[:, (b * 16 + jt) * 128:
                                          (b * 16 + jt) * 128 + 128],
                                    aT[:],
                                    start=(jt == 0), stop=(jt == 15))
                            oc0 = b * T + ic * 512
                            nc.vector.tensor_copy(
                                outT[:, oc0:oc0 + 512], psav[:])
                            # projection for this chunk
                            for it in range(4):
                                i_tile = ic * 4 + it
                                col = b * 16 + i_tile
                                i0 = b * T + i_tile * 128
                                fin = finp.tile([128, D], F32, tag="fin")
                                for nt in range(2):
                                    psp = ps_pr.tile([128, 512], F32,
                                                     tag="psp")
                                    nc.tensor.matmul(
                                        psp[:],
                                        outT[:, i0:i0 + 128],
                                        wout_sb[:, _ts(nt, 512)],
                                        start=True, stop=True)
                                    nc.vector.tensor_scalar_mul(
                                        fin[:, _ts(nt, 512)], psp[:],
                                        recip_all[:, col:col + 1])
                                nc.sync.dma_start(
                                    partial[i_tile * 128 + b * T:
                                            i_tile * 128 + b * T + 128, :],
                                    fin[:])
    nc.compile()
    return nc


# ======================= host side =======================

def _rn_tf32(a):
    u = np.ascontiguousarray(a, dtype=np.float32).view(np.uint32).astype(np.uint64)
    u = (u + 0xFFF + ((u >> 13) & 1)) & 0xFFFFE000
    return u.astype(np.uint32).view(np.float32)


def make_in_maps(inputs):
    x = np.asarray(inputs["x"], np.float32)
    Wqkv = np.asarray(inputs["Wqkv"], np.float32)
    bqkv = np.asarray(inputs["bqkv"], np.float32)
    Wout = np.asarray(inputs["Wout"], np.float32)
    centers = np.asarray(inputs["splat_centers"], np.float32)
    log_scales = np.asarray(inputs["splat_log_scales"], np.float64)
    log_amps = np.asarray(inputs["splat_log_amplitudes"], np.float64)
    temp = float(np.asarray(inputs["temperature"]).reshape(-1)[0])

    scales = np.exp(log_scales)
    inv2s2 = 0.5 / (scales + EPS) ** 2                      # [H, S]
    c_sq = (centers.astype(np.float64) ** 2).sum(-1)        # [H, S]
    xTr = _rn_tf32(x.reshape(BT, D).T)
    ident = np.eye(128, dtype=np.float32)
    invt = np.full((128, 1), 1.0 / temp, np.float32)
    ones_col = np.ones((Dh, 1), np.float32)

    in_maps = []
    for h in range(H):
        w_h = np.concatenate([Wqkv[:, c * D + h * Dh: c * D + (h + 1) * Dh]
                              for c in range(3)], axis=1)
        b_h = np.stack([bqkv[c * D + h * Dh: c * D + (h + 1) * Dh]
                        for c in range(3)], axis=1)
        cqs_h = centers[h].T * (2.0 * inv2s2[h])[None, :]
        bias_q = (-inv2s2[h] * c_sq[h] + log_amps[h]).astype(np.float32)
        bias_k = (-inv2s2[h] * c_sq[h]).astype(np.float32)
        in_maps.append({
            "xT": xTr,
            "wqkv": _rn_tf32(w_h),
            "bqkv": np.ascontiguousarray(b_h, np.float32),
            "cqs": _rn_tf32(cqs_h),
            "neginv": _rn_tf32(-inv2s2[h][None, :]),
            "biasq": bias_q[:, None],
            "biask": bias_k[:, None],
            "invt": invt,
            "ones_col": ones_col,
            "ident": ident,
            "wout": _rn_tf32(Wout[h * Dh:(h + 1) * Dh, :]),
        })
    return in_maps


def assemble(results, inputs):
    bout = np.asarray(inputs["bout"], np.float32)
    att = np.empty((B, T, T, H), np.float32)
    acc = np.zeros((BT, D), np.float64)
    for h in range(H):
        att[..., h] = results[h]["att"]
        acc += results[h]["partial"]
    final = (acc + bout).astype(np.float32).reshape(B, T, D)
    return final, att


def kernel(**inputs):
    import runner
    nc = build_kernel()
    in_maps = make_in_maps(inputs)
    results, _ = runner.run_spmd(nc, in_maps, n_cores=H)
    return assemble(results, inputs)


if __name__ == "__main__":
    build_kernel()
    print("build OK")
